# revision 13
# baseline (speedup 1.0000x reference)
"""Two-layer LSTM (H=51) over [B=4096, T=256] on 8 NeuronCores.

Strategy: data-parallel over batch (512 per core). Per core, a skewed
software pipeline over T+2 phases: phase q computes layer-1 of step q,
layer-2 of step q-1, and the linear head of step q-2.

Both layers' gate matmuls are fused into ONE matmul per gate bank: they
share the same rhs (stk) and their lhsT column blocks target disjoint
output rows (l1 -> 0..50, l2 -> 64..114). Phase 0's spurious l2 output
is cancelled by re-zeroing h2/c2 right after phase 0.

The cell uses native Sigmoid/Tanh activations (both live in the same
activation table, so only one table load at startup):
    sf,si,so = sigmoid(z_f,i,o)   [one ACT op over 3 banks]
    tg       = tanh(z_g)          [ACT]
    u = sf*c ; v = si*tg ; c' = u+v    [plain tensor_tensor]
    tc = tanh(c')                 [ACT]
    h' = so*tc                    [tensor_tensor]
All elementwise tensors, weights and states are bfloat16: the DVE
tensor_tensor ops run in the 2x 16-bit mode (194ns vs 327ns) and
matmuls run 1 cycle/row at any moving size. PSUM accumulation is fp32.
"""

import numpy as np

H = 51
T_FULL = 256
B_FULL = 4096
N_CORES = 8

# Stk partition layout (stacked matmul rhs):
#   rows 0..50   : h1
#   rows 51..63  : junk (zero, weighted by zero)
#   rows 64..114 : h2
#   row 115      : ones (bias row, DMA-initialized)
#   row 116      : x_t (DMA per step)
ROW_H1 = 0
ROW_JUNK = 51
ROW_H2 = 64
ROW_ONES = 115
ROW_X = 116
K_STK = 117
# gate-row space of the elementwise ops: rows 0..50 layer1, 51..63 junk,
# 64..114 layer2
GP = 115

MW = GP  # matmul output width (zero-padded gate lhsT columns)


def _build_weights(W_ih1, W_hh1, b_ih1, b_hh1, W_ih2, W_hh2, b_ih2, b_hh2,
                   W_lin, b_lin):
    """Host-side packing of lhsT weight tiles.

    Returns WG [K_STK, 4*MW + 1] float64. Four fused gate lhsTs of width
    MW=115 (banks f, i, o, g), each combining layer-1 (output rows 0..50:
    W_hh1 at h1 rows, W_ih1 at the x row, b1 at the ones row) and layer-2
    (output rows 64..114: W_ih2 at h1 rows, W_hh2 at h2 rows, b2 at ones).
    Column 4*MW rows 64..115: [W_lin; b_lin] for the out head
    (lhsT partitions must match its rhs Stk[64:116] = [h2; ones]).
    """
    b1 = (b_ih1 + b_hh1).astype(np.float64)
    b2 = (b_ih2 + b_hh2).astype(np.float64)
    # reference gate order in the stacked 4H rows: i, f, g, o
    idx = {"i": np.arange(0, H), "f": np.arange(H, 2 * H),
           "g": np.arange(2 * H, 3 * H), "o": np.arange(3 * H, 4 * H)}
    # bank order: f, i, o (sigmoid, contiguous) then g (tanh)
    order = ["f", "i", "o", "g"]
    WG = np.zeros((K_STK, 4 * MW + 1), dtype=np.float64)
    for xi, gate in enumerate(order):
        r = idx[gate]
        c0 = xi * MW
        # layer 1 (output rows 0..50): z1 = W_ih1 @ x + b1 + W_hh1 @ h1
        col1 = slice(c0, c0 + H)
        WG[ROW_ONES, col1] = b1[r]
        WG[ROW_H1:ROW_H1 + H, col1] = W_hh1[r, :].T
        WG[ROW_X, col1] = W_ih1[r, 0]
        # layer 2 (output rows 64..114): z2 = W_ih2 @ h1 + b2 + W_hh2 @ h2
        col2 = slice(c0 + ROW_H2, c0 + ROW_H2 + H)
        WG[ROW_ONES, col2] = b2[r]
        WG[ROW_H1:ROW_H1 + H, col2] = W_ih2[r, :].T
        WG[ROW_H2:ROW_H2 + H, col2] = W_hh2[r, :].T
    # out head: lhsT must sit at the same partitions as its rhs Stk[64:116]
    # (= [h2 (51); ones]), so W_lin goes at rows 64..114 and b_lin at 115.
    WG[ROW_H2:ROW_H2 + H, 4 * MW] = W_lin[0, :]
    WG[ROW_ONES, 4 * MW] = float(np.asarray(b_lin).reshape(-1)[0])
    return WG


def build_core_kernel(T, B, groups=2, mode="bf16"):
    """Build the per-core Bass kernel. Inputs: xT [T+1, B], WG [K_STK, 461].
    Output: out_bt [B, T] (full linear head incl. b_lin)."""
    import concourse.bacc as bacc
    import concourse.mybir as mybir
    from concourse.tile import TileContext

    fp = mybir.dt.float32
    bf16 = mode == "bf16"
    dt_w = mybir.dt.bfloat16 if bf16 else fp
    dt_e = dt_w
    Bg = B // groups

    nc = bacc.Bacc("TRN2", target_bir_lowering=False, debug=False)
    # xT row 0 is a host-prepended row of ones (feeds the bias row of Stk);
    # rows 1..T are input.T
    xT = nc.dram_tensor("xT", [T + 1, B], dt_w, kind="ExternalInput")
    WG = nc.dram_tensor("WG", [K_STK, 4 * MW + 1], dt_w, kind="ExternalInput")
    out_bt = nc.dram_tensor("out_bt", [B, T], fp, kind="ExternalOutput")

    C = min(128, T)  # output columns buffered in PSUM between flushes
    assert T % C == 0
    assert (B // groups) % 128 == 0, "batch per group must be a multiple of 128"

    with TileContext(nc) as tc:
        with (
            tc.tile_pool(name="persist", bufs=1) as persist,
            tc.tile_pool(name="gpsum", bufs=1, space="PSUM") as gpsum,
            tc.tile_pool(name="opsum", bufs=1, space="PSUM") as opsum,
            tc.tile_pool(name="temps", bufs=3) as temps,
            tc.tile_pool(name="ostage", bufs=2) as ostage,
        ):
            wg = persist.tile([K_STK, 4 * MW + 1], dt_w)
            nc.sync.dma_start(out=wg, in_=WG[:, :])

            nchunk = Bg // 128
            stks, cts, gps, pos = [], [], [], []
            for g in range(groups):
                stk = persist.tile([K_STK, Bg], dt_w, tag=f"stk{g}")
                ct = persist.tile([GP, Bg], dt_e, tag=f"ct{g}")
                gp = gpsum.tile([GP, 4 * Bg], fp, tag=f"gp{g}")
                nc.vector.memset(stk[:, :].bitcast(fp), 0.0)
                nc.sync.dma_start(out=stk[ROW_ONES:ROW_ONES + 1, :],
                                  in_=xT[0:1, g * Bg:(g + 1) * Bg])
                nc.vector.memset(ct[:, :].bitcast(fp), 0.0)
                stks.append(stk)
                cts.append(ct)
                gps.append(gp)
                pos.append(opsum.tile([128, nchunk * C], fp, tag=f"po{g}",
                                      name=f"po{g}"))

            mult = mybir.AluOpType.mult
            add = mybir.AluOpType.add
            tanh = mybir.ActivationFunctionType.Tanh
            sigm = mybir.ActivationFunctionType.Sigmoid

            for q in range(T + 2):
                mm = q <= T
                # ---- x load for step q + fused gate matmuls (all 4 banks,
                # both layers in one matmul each; at q==T the l1 half reads
                # stale x and produces garbage h1_T/c1_T, which nothing
                # consumes).
                for g in range(groups):
                    stk, gp = stks[g], gps[g]
                    cols = slice(g * Bg, (g + 1) * Bg)
                    if q < T:
                        nc.sync.dma_start(out=stk[ROW_X:ROW_X + 1, :],
                                          in_=xT[q + 1:q + 2, cols])
                    if mm:
                        rhs = stk[0:K_STK, :]
                        for xi in range(4):
                            nc.tensor.matmul(
                                gp[0:GP, xi * Bg:(xi + 1) * Bg],
                                wg[0:K_STK, xi * MW:(xi + 1) * MW],
                                rhs, start=True, stop=True)
                # ---- out head for step t = q-2: out[:, t] column
                if q >= 2:
                    t = q - 2
                    tc_col = t % C
                    for g in range(groups):
                        stk = stks[g]
                        for k in range(nchunk):
                            nc.tensor.matmul(
                                pos[g][:, k * C + tc_col:k * C + tc_col + 1],
                                stk[64:116, k * 128:(k + 1) * 128],
                                wg[64:116, 4 * MW:4 * MW + 1],
                                start=True, stop=True)
                    if tc_col == C - 1:  # flush epoch
                        t0 = t - (C - 1)
                        for g in range(groups):
                            for k in range(nchunk):
                                st = ostage.tile([128, C], fp, tag=f"os{g}_{k}")
                                nc.vector.tensor_copy(
                                    st, pos[g][:, k * C:(k + 1) * C])
                                row0 = g * Bg + k * 128
                                nc.sync.dma_start(
                                    out=out_bt[row0:row0 + 128, t0:t0 + C],
                                    in_=st)
                # ---- elementwise chain per group. Banks: 0=f, 1=i, 2=o, 3=g.
                if mm:
                    for g in range(groups):
                        sg_t = temps.tile([GP, 4 * Bg], dt_e, tag=f"sg{g}")
                        # sigmoid over f,i,o banks; tanh over the g bank
                        nc.scalar.activation(sg_t[:, 0:3 * Bg],
                                             gps[g][0:GP, 0:3 * Bg], sigm)
                        nc.scalar.activation(sg_t[:, 3 * Bg:4 * Bg],
                                             gps[g][0:GP, 3 * Bg:4 * Bg], tanh)
                        sf = sg_t[:, 0 * Bg:1 * Bg]
                        si = sg_t[:, 1 * Bg:2 * Bg]
                        so = sg_t[:, 2 * Bg:3 * Bg]
                        tg = sg_t[:, 3 * Bg:4 * Bg]
                        v = temps.tile([GP, Bg], dt_e, tag=f"v{g}")
                        u = temps.tile([GP, Bg], dt_e, tag=f"u{g}")
                        tcl = temps.tile([GP, Bg], dt_e, tag=f"tc{g}")
                        # u = sf*c ; v = si*tg ; c' = u+v
                        nc.vector.tensor_tensor(u, sf, cts[g][:, :], mult)
                        nc.vector.tensor_tensor(v, si, tg, mult)
                        nc.vector.tensor_tensor(cts[g][:, :], u, v, add)
                        # tc = tanh(c') ; h' = so*tc
                        nc.scalar.activation(tcl, cts[g][:, :], tanh)
                        nc.vector.tensor_tensor(
                            stks[g][ROW_H1:ROW_H1 + GP, :], so, tcl, mult)
                if q == 0:
                    # cancel phase 0's spurious l2 output: h2/c2 must enter
                    # phase 1 as zero.
                    for g in range(groups):
                        nc.vector.memset(
                            stks[g][ROW_H2:ROW_H2 + H, :].bitcast(fp), 0.0)
                        nc.vector.memset(
                            cts[g][ROW_H2:ROW_H2 + H, :].bitcast(fp), 0.0)
    nc.compile()
    return nc


_NC_CACHE = {}


def _get_nc(T, B, groups, mode):
    key = (T, B, groups, mode)
    if key not in _NC_CACHE:
        _NC_CACHE[key] = build_core_kernel(T, B, groups, mode)
    return _NC_CACHE[key]


def kernel(input, W_ih1, W_hh1, b_ih1, b_hh1, W_ih2, W_hh2, b_ih2, b_hh2,
           W_lin, b_lin, _groups=2, _mode="bf16"):
    import ml_dtypes
    from concourse import bass_utils

    input = np.asarray(input, dtype=np.float32)
    B, T = input.shape
    Bc = B // N_CORES
    WG = _build_weights(np.asarray(W_ih1, np.float64), np.asarray(W_hh1, np.float64),
                        np.asarray(b_ih1, np.float64), np.asarray(b_hh1, np.float64),
                        np.asarray(W_ih2, np.float64), np.asarray(W_hh2, np.float64),
                        np.asarray(b_ih2, np.float64), np.asarray(b_hh2, np.float64),
                        np.asarray(W_lin, np.float64), np.asarray(b_lin, np.float64))
    # row 0 = ones (bias row), rows 1..T = input.T
    xT = np.concatenate([np.ones((1, B), np.float64),
                         np.asarray(input, np.float64).T])
    nc = _get_nc(T, Bc, _groups, _mode)
    host_dt = ml_dtypes.bfloat16 if _mode == "bf16" else np.float32
    xTh = np.ascontiguousarray(xT).astype(host_dt)
    WGh = np.ascontiguousarray(WG).astype(host_dt)
    in_maps = [
        {"xT": np.ascontiguousarray(xTh[:, c * Bc:(c + 1) * Bc]), "WG": WGh}
        for c in range(N_CORES)
    ]
    res = bass_utils.run_bass_kernel_spmd(
        nc, in_maps, core_ids=list(range(N_CORES)), trace=False)
    outs = [res.results[c]["out_bt"] for c in range(N_CORES)]  # [Bc, T] each
    out = np.concatenate(outs, axis=0)  # [B, T]
    return out.astype(np.float32)


# revision 17
# speedup vs baseline: 1.0105x; 1.0105x over previous
"""Two-layer LSTM (H=51) over [B=4096, T=256] on 8 NeuronCores.

Strategy: data-parallel over batch (512 per core). Per core, a skewed
software pipeline over T+2 phases: phase q computes layer-1 of step q,
layer-2 of step q-1, and the linear head of step q-2.

Both layers' gate matmuls are fused into ONE matmul per gate bank: they
share the same rhs (stk) and their lhsT column blocks target disjoint
output rows (l1 -> 0..50, l2 -> 64..114). Phase 0's spurious l2 output
is cancelled by re-zeroing h2/c2 right after phase 0.

The cell uses native Sigmoid/Tanh activations (both live in the same
activation table, so only one table load at startup):
    sf,si,so = sigmoid(z_f,i,o)   [one ACT op over 3 banks]
    tg       = tanh(z_g)          [ACT]
    u = sf*c ; v = si*tg ; c' = u+v    [plain tensor_tensor]
    tc = tanh(c')                 [ACT]
    h' = so*tc                    [tensor_tensor]
All elementwise tensors, weights and states are bfloat16: the DVE
tensor_tensor ops run in the 2x 16-bit mode (194ns vs 327ns) and
matmuls run 1 cycle/row at any moving size. PSUM accumulation is fp32.
"""

import numpy as np

H = 51
T_FULL = 256
B_FULL = 4096
N_CORES = 8

# Stk partition layout (stacked matmul rhs):
#   rows 0..50   : h1
#   rows 51..63  : junk (zero, weighted by zero)
#   rows 64..114 : h2
#   row 115      : ones (bias row, DMA-initialized)
#   row 116      : x_t (DMA per step)
ROW_H1 = 0
ROW_JUNK = 51
ROW_H2 = 64
ROW_ONES = 115
ROW_X = 116
K_STK = 117
# gate-row space of the elementwise ops: rows 0..50 layer1, 51..63 junk,
# 64..114 layer2
GP = 115

MW = GP  # matmul output width (zero-padded gate lhsT columns)


def _build_weights(W_ih1, W_hh1, b_ih1, b_hh1, W_ih2, W_hh2, b_ih2, b_hh2,
                   W_lin, b_lin):
    """Host-side packing of lhsT weight tiles.

    Returns WG [K_STK, 4*MW + 1] float64. Four fused gate lhsTs of width
    MW=115 (banks f, i, o, g), each combining layer-1 (output rows 0..50:
    W_hh1 at h1 rows, W_ih1 at the x row, b1 at the ones row) and layer-2
    (output rows 64..114: W_ih2 at h1 rows, W_hh2 at h2 rows, b2 at ones).
    Column 4*MW rows 64..115: [W_lin; b_lin] for the out head
    (lhsT partitions must match its rhs Stk[64:116] = [h2; ones]).
    """
    b1 = (b_ih1 + b_hh1).astype(np.float64)
    b2 = (b_ih2 + b_hh2).astype(np.float64)
    # reference gate order in the stacked 4H rows: i, f, g, o
    idx = {"i": np.arange(0, H), "f": np.arange(H, 2 * H),
           "g": np.arange(2 * H, 3 * H), "o": np.arange(3 * H, 4 * H)}
    # bank order: g (tanh) first — its matmul and tanh run while the f,i,o
    # matmuls finish; then f,i,o (sigmoid, contiguous)
    order = ["g", "f", "i", "o"]
    WG = np.zeros((K_STK, 4 * MW + 1), dtype=np.float64)
    for xi, gate in enumerate(order):
        r = idx[gate]
        c0 = xi * MW
        # layer 1 (output rows 0..50): z1 = W_ih1 @ x + b1 + W_hh1 @ h1
        col1 = slice(c0, c0 + H)
        WG[ROW_ONES, col1] = b1[r]
        WG[ROW_H1:ROW_H1 + H, col1] = W_hh1[r, :].T
        WG[ROW_X, col1] = W_ih1[r, 0]
        # layer 2 (output rows 64..114): z2 = W_ih2 @ h1 + b2 + W_hh2 @ h2
        col2 = slice(c0 + ROW_H2, c0 + ROW_H2 + H)
        WG[ROW_ONES, col2] = b2[r]
        WG[ROW_H1:ROW_H1 + H, col2] = W_ih2[r, :].T
        WG[ROW_H2:ROW_H2 + H, col2] = W_hh2[r, :].T
    # out head: lhsT must sit at the same partitions as its rhs Stk[64:116]
    # (= [h2 (51); ones]), so W_lin goes at rows 64..114 and b_lin at 115.
    WG[ROW_H2:ROW_H2 + H, 4 * MW] = W_lin[0, :]
    WG[ROW_ONES, 4 * MW] = float(np.asarray(b_lin).reshape(-1)[0])
    return WG


def build_core_kernel(T, B, groups=2, mode="bf16", skew=8):
    """Build the per-core Bass kernel. Inputs: xT [T+1, B], WG [K_STK, 461].
    Output: out_bt [B, T] (full linear head incl. b_lin)."""
    import concourse.bacc as bacc
    import concourse.mybir as mybir
    from concourse.tile import TileContext

    fp = mybir.dt.float32
    bf16 = mode == "bf16"
    dt_w = mybir.dt.bfloat16 if bf16 else fp
    dt_e = dt_w
    Bg = B // groups

    nc = bacc.Bacc("TRN2", target_bir_lowering=False, debug=False)
    # xT row 0 is a host-prepended row of ones (feeds the bias row of Stk);
    # rows 1..T are input.T
    xT = nc.dram_tensor("xT", [T + 1, B], dt_w, kind="ExternalInput")
    WG = nc.dram_tensor("WG", [K_STK, 4 * MW + 1], dt_w, kind="ExternalInput")
    out_bt = nc.dram_tensor("out_bt", [B, T], fp, kind="ExternalOutput")

    C = min(128, T)  # output columns buffered in PSUM between flushes
    assert T % C == 0
    assert (B // groups) % 128 == 0, "batch per group must be a multiple of 128"

    with TileContext(nc) as tc:
        with (
            tc.tile_pool(name="persist", bufs=1) as persist,
            tc.tile_pool(name="gpsum", bufs=1, space="PSUM") as gpsum,
            tc.tile_pool(name="opsum", bufs=1, space="PSUM") as opsum,
            tc.tile_pool(name="temps", bufs=3) as temps,
            tc.tile_pool(name="ostage", bufs=2) as ostage,
        ):
            wg = persist.tile([K_STK, 4 * MW + 1], dt_w)
            nc.sync.dma_start(out=wg, in_=WG[:, :])

            nchunk = Bg // 128
            stks, cts, gps, pos = [], [], [], []
            for g in range(groups):
                stk = persist.tile([K_STK, Bg], dt_w, tag=f"stk{g}")
                ct = persist.tile([GP, Bg], dt_e, tag=f"ct{g}")
                gp = gpsum.tile([GP, 4 * Bg], fp, tag=f"gp{g}")
                nc.vector.memset(stk[:, :].bitcast(fp), 0.0)
                nc.sync.dma_start(out=stk[ROW_ONES:ROW_ONES + 1, :],
                                  in_=xT[0:1, g * Bg:(g + 1) * Bg])
                nc.vector.memset(ct[:, :].bitcast(fp), 0.0)
                stks.append(stk)
                cts.append(ct)
                gps.append(gp)
                pos.append(opsum.tile([128, nchunk * C], fp, tag=f"po{g}",
                                      name=f"po{g}"))

            mult = mybir.AluOpType.mult
            add = mybir.AluOpType.add
            tanh = mybir.ActivationFunctionType.Tanh
            sigm = mybir.ActivationFunctionType.Sigmoid

            for q in range(T + 2):
                mm = q <= T
                # ---- x load for step q + fused gate matmuls (all 4 banks,
                # both layers in one matmul each; at q==T the l1 half reads
                # stale x and produces garbage h1_T/c1_T, which nothing
                # consumes).
                for g in range(groups):
                    stk, gp = stks[g], gps[g]
                    cols = slice(g * Bg, (g + 1) * Bg)
                    if q < T:
                        nc.sync.dma_start(out=stk[ROW_X:ROW_X + 1, :],
                                          in_=xT[q + 1:q + 2, cols])
                    if mm:
                        rhs = stk[0:K_STK, :]
                        for xi in range(4):
                            nc.tensor.matmul(
                                gp[0:GP, xi * Bg:(xi + 1) * Bg],
                                wg[0:K_STK, xi * MW:(xi + 1) * MW],
                                rhs, start=True, stop=True)
                # ---- out head for step t = q-2: out[:, t] column
                if q >= 2:
                    t = q - 2
                    tc_col = t % C
                    for g in range(groups):
                        stk = stks[g]
                        for k in range(nchunk):
                            nc.tensor.matmul(
                                pos[g][:, k * C + tc_col:k * C + tc_col + 1],
                                stk[64:116, k * 128:(k + 1) * 128],
                                wg[64:116, 4 * MW:4 * MW + 1],
                                start=True, stop=True)
                    if tc_col == C - 1:  # flush epoch
                        t0 = t - (C - 1)
                        for g in range(groups):
                            for k in range(nchunk):
                                st = ostage.tile([128, C], fp, tag=f"os{g}_{k}")
                                nc.vector.tensor_copy(
                                    st, pos[g][:, k * C:(k + 1) * C])
                                row0 = g * Bg + k * 128
                                nc.sync.dma_start(
                                    out=out_bt[row0:row0 + 128, t0:t0 + C],
                                    in_=st)
                # ---- elementwise chain per group. Banks: 0=g, 1=f, 2=i, 3=o.
                if mm:
                    for g in range(groups):
                        sg_t = temps.tile([GP, 4 * Bg], dt_e, tag=f"sg{g}")
                        # tanh over the g bank first (its matmul is emitted
                        # first); sigmoid over f,i,o banks
                        nc.scalar.activation(sg_t[:, 0:Bg],
                                             gps[g][0:GP, 0:Bg], tanh)
                        nc.scalar.activation(sg_t[:, Bg:4 * Bg],
                                             gps[g][0:GP, Bg:4 * Bg], sigm)
                        tg = sg_t[:, 0 * Bg:1 * Bg]
                        sf = sg_t[:, 1 * Bg:2 * Bg]
                        si = sg_t[:, 2 * Bg:3 * Bg]
                        so = sg_t[:, 3 * Bg:4 * Bg]
                        v = temps.tile([GP, Bg], dt_e, tag=f"v{g}")
                        u = temps.tile([GP, Bg], dt_e, tag=f"u{g}")
                        tcl = temps.tile([GP, Bg], dt_e, tag=f"tc{g}")
                        # u = sf*c ; v = si*tg ; c' = u+v
                        nc.vector.tensor_tensor(u, sf, cts[g][:, :], mult)
                        nc.vector.tensor_tensor(v, si, tg, mult)
                        nc.vector.tensor_tensor(cts[g][:, :], u, v, add)
                        # tc = tanh(c') ; h' = so*tc
                        nc.scalar.activation(tcl, cts[g][:, :], tanh)
                        nc.vector.tensor_tensor(
                            stks[g][ROW_H1:ROW_H1 + GP, :], so, tcl, mult)
                if q == 0:
                    # cancel phase 0's spurious l2 output: h2/c2 must enter
                    # phase 1 as zero.
                    for g in range(groups):
                        nc.vector.memset(
                            stks[g][ROW_H2:ROW_H2 + H, :].bitcast(fp), 0.0)
                        nc.vector.memset(
                            cts[g][ROW_H2:ROW_H2 + H, :].bitcast(fp), 0.0)
                    # de-synchronize the groups: a chain of exact identity
                    # multiplies delays group g's c-path by ~g*P/groups so the
                    # per-group pipelines settle into collision-free phase
                    # offsets instead of lockstep (where each group's chain
                    # queues behind the other's ACT ops every step).
                    for g in range(1, groups):
                        for _ in range(g * skew):
                            nc.vector.tensor_scalar_mul(cts[g][:, :],
                                                        cts[g][:, :], 1.0)
    nc.compile()
    return nc


_NC_CACHE = {}


def _get_nc(T, B, groups, mode, skew=8):
    key = (T, B, groups, mode, skew)
    if key not in _NC_CACHE:
        _NC_CACHE[key] = build_core_kernel(T, B, groups, mode, skew)
    return _NC_CACHE[key]


def kernel(input, W_ih1, W_hh1, b_ih1, b_hh1, W_ih2, W_hh2, b_ih2, b_hh2,
           W_lin, b_lin, _groups=2, _mode="bf16", _skew=8):
    import ml_dtypes
    from concourse import bass_utils

    input = np.asarray(input, dtype=np.float32)
    B, T = input.shape
    Bc = B // N_CORES
    WG = _build_weights(np.asarray(W_ih1, np.float64), np.asarray(W_hh1, np.float64),
                        np.asarray(b_ih1, np.float64), np.asarray(b_hh1, np.float64),
                        np.asarray(W_ih2, np.float64), np.asarray(W_hh2, np.float64),
                        np.asarray(b_ih2, np.float64), np.asarray(b_hh2, np.float64),
                        np.asarray(W_lin, np.float64), np.asarray(b_lin, np.float64))
    # row 0 = ones (bias row), rows 1..T = input.T
    xT = np.concatenate([np.ones((1, B), np.float64),
                         np.asarray(input, np.float64).T])
    nc = _get_nc(T, Bc, _groups, _mode, _skew)
    host_dt = ml_dtypes.bfloat16 if _mode == "bf16" else np.float32
    xTh = np.ascontiguousarray(xT).astype(host_dt)
    WGh = np.ascontiguousarray(WG).astype(host_dt)
    in_maps = [
        {"xT": np.ascontiguousarray(xTh[:, c * Bc:(c + 1) * Bc]), "WG": WGh}
        for c in range(N_CORES)
    ]
    res = bass_utils.run_bass_kernel_spmd(
        nc, in_maps, core_ids=list(range(N_CORES)), trace=False)
    outs = [res.results[c]["out_bt"] for c in range(N_CORES)]  # [Bc, T] each
    out = np.concatenate(outs, axis=0)  # [B, T]
    return out.astype(np.float32)
